# revision 11
# baseline (speedup 1.0000x reference)
"""CBAM block (channel + spatial attention) Trainium2 Bass kernel.

Problem: x [32, 56, 56, 256] f32; data-parallel over batch across 8 NeuronCores
(4 images per core).  Everything is hardcoded for these shapes.

Per-core dataflow (B=4 images, each [3136(hw), 256(c)] f32, SBUF-resident):

  Layout: flat row n of an image maps to (partition p, block t) as
    group A: p in [0, 64),    t in [0, 25): n = 25*p + t
    group B: p in [64, 128),  t in [0, 24): n = 1600 + 24*(p-64) + t
  so each partition holds a CONTIGUOUS run of rows; every in/out DMA moves
  24-25 KB contiguous runs per partition, and the spatial maps [p, t] are
  flat-n contiguous per partition (cheap DRAM roundtrip for the conv
  rearrange).  Per-block op shapes are identical to a 128x(24)+64x1 split:
  t in [0, 24) covers all 128 partitions, t = 24 covers partitions [0, 64).

  Phase A (channel attention, per image):
    - max over hw: DVE tensor_reduce over t + half-block max -> acc [128, 256];
      2 PE transposes + DVE max over the transposed chunks -> statsT[:, :, 1]
      (channel-on-partition layout, no gpsimd all-reduce).
    - sum over hw: PE one-hot matmuls accumulated over the 25 blocks -> ACT
      scale 1/HW -> 2 PE transposes -> statsT[:, :, 0].
    - MLP W1/relu/W2, sigmoid(colA + colM + 2*b2) -> caT [128, 2] ->
      PE transpose + ones-broadcast matmul -> bca[:, b, :] [128, 256].
  Toeplitz bands (after phase A issue, consumed by the conv):
    t_sb[h_in, ch*7+dwi, h_out] = conv_w[h_in-h_out+3, dwi, ch], built on-chip:
    GPSIMD scales ident56 by each broadcast conv weight (diag matrices), PE
    accumulates diag @ dmask into PSUM per (ch, dwi) band.
  Phase B1 (per image, per block):
    - DVE tensor_tensor_reduce: xr = x * bca in place, accum max over c -> maxc.
    - sum over c -> sumc: even blocks ACT copy+accum_out, odd blocks DVE
      tensor_reduce (engine balance).
  Phase B2 (per image): maps [p, t] -> DRAM flat n -> [h, w] tiles; conv as 14
    accumulated PE matmuls (Toeplitz over h, shift over w); ACT sigmoid; back
    via DRAM flat -> saf [p, t].
  Phase B3 (per image): out = xr * sa per-partition scalar per block, split
    ACT/GPSIMD; 2 contiguous out-DMAs.
"""

import os

import numpy as np

import concourse.bass as bass
import concourse.bacc as bacc
import concourse.bass_isa as bass_isa
import concourse.tile as tile
from concourse import mybir
from concourse.bass_utils import run_bass_kernel_spmd

F32 = mybir.dt.float32
AX = mybir.AxisListType
OP = mybir.AluOpType
ACT = mybir.ActivationFunctionType

P = 128          # partitions
NB = 25          # blocks in group A (group B has 24)
NBF = 24         # full-width blocks
HALF = 64        # partitions in group A / valid rows in block 24
C = 256          # channels
HW = 3136        # 56*56
GA = 1600        # rows in group A (64 * 25)
NIMG = 4         # images per core
NCORES = 8

_CACHE: dict = {}


def _pp(t: int) -> int:
    return P if t < NBF else HALF


def _build_nc() -> bass.Bass:
    nc = bacc.Bacc()

    x_d = nc.dram_tensor("x", [NIMG, 56, 56, C], F32, kind="ExternalInput")
    w1_d = nc.dram_tensor("w1", [C, 16], F32, kind="ExternalInput")
    b1_d = nc.dram_tensor("b1", [16], F32, kind="ExternalInput")
    w2_d = nc.dram_tensor("w2", [16, C], F32, kind="ExternalInput")
    b2_d = nc.dram_tensor("b2", [C], F32, kind="ExternalInput")
    cw_d = nc.dram_tensor("conv_w", [7, 7, 2, 1], F32, kind="ExternalInput")
    out_d = nc.dram_tensor("out", [NIMG, 56, 56, C], F32, kind="ExternalOutput")

    ident_d = nc.inline_tensor(np.eye(128, dtype=np.float32), name="ident128")

    # D[a, dhi, b] = 1 iff a - b == dhi - 3 (0/1 diagonal masks; the Toeplitz
    # band over h for tap dhi)
    dmask_np = np.zeros((56, 7, 56), dtype=np.float32)
    for di in range(7):
        for a in range(56):
            b = a - (di - 3)
            if 0 <= b < 56:
                dmask_np[a, di, b] = 1.0
    dmask_d = nc.inline_tensor(dmask_np, name="dmask")

    x_hwc = x_d[:].rearrange("b h w c -> b (h w) c")
    out_hwc = out_d[:].rearrange("b h w c -> b (h w) c")

    with tile.TileContext(nc) as tc:
        import contextlib

        with contextlib.ExitStack() as ctx:
            cpool = ctx.enter_context(tc.tile_pool(name="cpool", bufs=1))
            xpool = ctx.enter_context(tc.tile_pool(name="xpool", bufs=1))
            work = ctx.enter_context(tc.tile_pool(name="work", bufs=3))
            small = ctx.enter_context(tc.tile_pool(name="small", bufs=3))
            psA = ctx.enter_context(tc.tile_pool(name="psA", bufs=2, space="PSUM"))
            psB = ctx.enter_context(tc.tile_pool(name="psB", bufs=2, space="PSUM"))
            psM = ctx.enter_context(tc.tile_pool(name="psM", bufs=2, space="PSUM"))
            dpool = ctx.enter_context(tc.tile_pool(name="dpool", bufs=2, space="DRAM"))

            # ---------------- constants & weights ----------------
            ident = cpool.tile([128, 128], F32)
            nc.sync.dma_start(out=ident, in_=ident_d[:])

            w1_sb = cpool.tile([128, 2, 16], F32)
            nc.sync.dma_start(out=w1_sb, in_=w1_d[:].rearrange("(j p) m -> p j m", p=128))
            w2_sb = cpool.tile([16, 2, 128], F32)
            nc.sync.dma_start(out=w2_sb, in_=w2_d[:].rearrange("k (j m) -> k j m", j=2))
            b1_sb = cpool.tile([16, 1], F32)
            nc.sync.dma_start(out=b1_sb, in_=b1_d[:].rearrange("(p o) -> p o", o=1))
            b2_sb = cpool.tile([128, 2], F32)
            nc.sync.dma_start(out=b2_sb, in_=b2_d[:].rearrange("(j p) -> p j", p=128))
            b2x2 = cpool.tile([128, 2], F32)
            nc.scalar.activation(out=b2x2, in_=b2_sb, func=ACT.Copy, scale=2.0)

            oh2 = cpool.tile([128, 2], F32)
            nc.vector.memset(oh2[:, 0:1], 1.0)
            nc.vector.memset(oh2[:, 1:2], 0.0)
            ones_r = cpool.tile([1, 128], F32)
            nc.vector.memset(ones_r, 1.0)

            dmask_sb = cpool.tile([56, 7, 56], F32)
            nc.sync.dma_start(out=dmask_sb, in_=dmask_d[:])
            cw_row = cpool.tile([1, 98], F32)
            nc.sync.dma_start(
                out=cw_row, in_=cw_d[:].rearrange("a b c o -> o (a b c)")
            )
            # broadcast the 98 conv weights to 56 partitions
            pcw = psM.tile([56, 98], F32, tag="mlp")
            nc.tensor.matmul(
                pcw, lhsT=ones_r[:, 0:56], rhs=cw_row, start=True, stop=True
            )
            cwb = cpool.tile([56, 98], F32)
            nc.scalar.copy(out=cwb, in_=pcw)

            # ---------------- big SBUF state ----------------
            X = xpool.tile([P, NIMG, NB, C], F32)
            bca = cpool.tile([P, NIMG, C], F32)
            maps = cpool.tile([P, NIMG, 2, NB], F32)  # [., ., 0=sumc 1=maxc, .]
            saf = cpool.tile([P, NIMG, NB], F32)
            scr = cpool.tile([P, C], F32)  # ACT dummy-copy target for accum

            # GPSIMD: build the 98 diag(conv_w) matrices early (consumed by
            # the Toeplitz PE matmuls issued after phase A)
            diags = cpool.tile([56, 98, 56], F32)
            for i in range(98):
                nc.gpsimd.tensor_scalar_mul(
                    out=diags[:, i, :],
                    in0=ident[0:56, 0:56],
                    scalar1=cwb[:, i : i + 1],
                )

            # PE warm-up matmuls touching constant lhsT sources so that later
            # matmuls don't accumulate one sync-wait per constant tensor.
            pwu = psM.tile([128, 4], F32, tag="mlp")
            nc.tensor.matmul(pwu[0:2, 0:2], lhsT=oh2, rhs=oh2, start=True, stop=True)
            nc.tensor.matmul(
                pwu[0:4, 0:4],
                lhsT=ident[:, 0:4],
                rhs=ident[:, 0:4],
                start=True,
                stop=True,
            )
            nc.tensor.matmul(
                pwu[0:128, 0:1],
                lhsT=ones_r.rearrange("p m -> p m"),
                rhs=ones_r[:, 0:1],
                start=True,
                stop=True,
            )
            nc.tensor.matmul(
                pwu[0:4, 0:4],
                lhsT=dmask_sb[:, 0, 0:4],
                rhs=dmask_sb[:, 0, 0:4],
                start=True,
                stop=True,
            )
            nc.tensor.matmul(
                pwu[0:4, 0:4],
                lhsT=w1_sb[:, 0, 0:4],
                rhs=w1_sb[:, 0, 0:4],
                start=True,
                stop=True,
            )
            nc.tensor.matmul(
                pwu[0:4, 0:4],
                lhsT=w2_sb[:, 0, 0:4],
                rhs=w2_sb[:, 0, 0:4],
                start=True,
                stop=True,
            )

            # ---------------- DMA in (contiguous runs per partition) --------
            for b in range(NIMG):
                nc.sync.dma_start(
                    out=X[0:HALF, b, 0:NB, :],
                    in_=x_hwc[b, 0:GA, :].rearrange("(p t) c -> p t c", t=NB),
                )
                nc.sync.dma_start(
                    out=X[HALF:P, b, 0:NBF, :],
                    in_=x_hwc[b, GA:HW, :].rearrange("(p t) c -> p t c", t=NBF),
                )

            # ---------------- phase A (channel attention) per image ---------
            def phase_a(b):
                # max over hw: contiguous log-tree of pairwise maxes over the
                # 24 full blocks (3D APs, c innermost), then the half block;
                # cross-partition via PE transposes + DVE max afterwards
                m12 = work.tile([P, 12, C], F32, tag="m12", bufs=1)
                m6 = work.tile([P, 6, C], F32, tag="m6", bufs=1)
                m3 = work.tile([P, 3, C], F32, tag="m3", bufs=1)
                acc = work.tile([P, C], F32, tag="acc")
                nc.vector.tensor_max(
                    out=m12, in0=X[:, b, 0:12, :], in1=X[:, b, 12:NBF, :]
                )
                nc.vector.tensor_max(
                    out=m6, in0=m12[:, 0:6, :], in1=m12[:, 6:12, :]
                )
                nc.vector.tensor_max(
                    out=m3, in0=m6[:, 0:3, :], in1=m6[:, 3:6, :]
                )
                nc.vector.tensor_max(
                    out=acc, in0=m3[:, 0, :], in1=m3[:, 1, :]
                )
                nc.vector.tensor_max(out=acc, in0=acc, in1=m3[:, 2, :])
                nc.vector.tensor_max(
                    out=acc[0:HALF], in0=acc[0:HALF], in1=X[0:HALF, b, NBF, :]
                )
                pmaxT = psM.tile([128, 2, 128], F32, tag="mlp")
                for j in range(2):
                    nc.tensor.transpose(
                        pmaxT[:, j, :], acc[:, j * 128 : (j + 1) * 128], ident
                    )
                statsT = small.tile([128, 2, 2], F32, tag="statsT")
                nc.vector.tensor_reduce(
                    out=statsT[:, :, 1:2], in_=pmaxT, axis=AX.X, op=OP.max
                )

                # sum over hw on PE (lands on psum partition row 0)
                ps = psA.tile([2, C], F32, tag="ps_sum")
                for t in range(NBF):
                    nc.tensor.matmul(
                        ps, lhsT=oh2, rhs=X[:, b, t, :], start=(t == 0), stop=False
                    )
                nc.tensor.matmul(
                    ps,
                    lhsT=oh2[0:HALF],
                    rhs=X[0:HALF, b, NBF, :],
                    start=False,
                    stop=True,
                )
                savg = small.tile([1, C], F32, tag="savg")
                nc.scalar.activation(
                    out=savg, in_=ps[0:1, :], func=ACT.Copy, scale=1.0 / HW
                )
                pavgT = psM.tile([128, 2, 1], F32, tag="mlp")
                for j in range(2):
                    nc.tensor.transpose(
                        pavgT[:, j, :],
                        savg[:, j * 128 : (j + 1) * 128],
                        ident[0:1, 0:1],
                    )
                nc.scalar.copy(out=statsT[:, :, 0:1], in_=pavgT)

                # MLP layer 1: h = relu(W1^T statsT + b1)
                ph = psM.tile([16, 2], F32, tag="mlp")
                for j in range(2):
                    nc.tensor.matmul(
                        ph,
                        lhsT=w1_sb[:, j, :],
                        rhs=statsT[:, j, :],
                        start=(j == 0),
                        stop=(j == 1),
                    )
                h_sb = small.tile([16, 2], F32, tag="h_sb")
                nc.scalar.activation(
                    out=h_sb, in_=ph, func=ACT.Relu, bias=b1_sb, scale=1.0
                )

                # layer 2 + combine + sigmoid -> caT [256] in 2 chunks
                caT = small.tile([128, 2], F32, tag="caT")
                for j in range(2):
                    pc = psM.tile([128, 2], F32, tag="mlp")
                    nc.tensor.matmul(
                        pc, lhsT=w2_sb[:, j, :], rhs=h_sb, start=True, stop=True
                    )
                    pc_sb = small.tile([128, 2], F32, tag="pc_sb")
                    nc.scalar.copy(out=pc_sb, in_=pc)
                    catmp = small.tile([128, 1], F32, tag="catmp")
                    nc.vector.tensor_add(
                        out=catmp, in0=pc_sb[:, 0:1], in1=pc_sb[:, 1:2]
                    )
                    nc.scalar.activation(
                        out=caT[:, j : j + 1],
                        in_=catmp,
                        func=ACT.Sigmoid,
                        bias=b2x2[:, j : j + 1],
                        scale=1.0,
                    )

                # broadcast ca over partitions: bca[:, b, :]
                pcr = psM.tile([1, 2, 128], F32, tag="mlp")
                for j in range(2):
                    nc.tensor.transpose(pcr[:, j, :], caT[:, j : j + 1], ident)
                ca_row = small.tile([1, 256], F32, tag="ca_row")
                nc.scalar.copy(out=ca_row, in_=pcr.rearrange("p j m -> p (j m)"))
                pbca = psB.tile([P, C], F32, tag="pbca")
                nc.tensor.matmul(pbca, lhsT=ones_r, rhs=ca_row, start=True, stop=True)
                nc.scalar.copy(out=bca[:, b, :], in_=pbca)

            # ---------------- phase B1: xr = x * ca, spatial stats ----------
            def phase_b1(b):
                for t in range(NB):
                    pp = _pp(t)
                    nc.vector.tensor_mul(
                        out=X[0:pp, b, t, :],
                        in0=X[0:pp, b, t, :],
                        in1=bca[0:pp, b, :],
                    )
                    nc.scalar.activation(
                        out=scr[0:pp, :],
                        in_=X[0:pp, b, t, :],
                        func=ACT.Copy,
                        accum_out=maps[0:pp, b, 0, t : t + 1],
                    )
                # max over c: one contiguous 3D reduce for the 24 full
                # blocks + one for the group-A extra block
                nc.vector.tensor_reduce(
                    out=maps[:, b, 1, 0:NBF],
                    in_=X[:, b, 0:NBF, :],
                    axis=AX.X,
                    op=OP.max,
                )
                nc.vector.tensor_reduce(
                    out=maps[0:HALF, b, 1, NBF : NBF + 1],
                    in_=X[0:HALF, b, NBF : NBF + 1, :],
                    axis=AX.X,
                    op=OP.max,
                )
                # mean = sum / C
                nc.scalar.activation(
                    out=maps[:, b, 0, :],
                    in_=maps[:, b, 0, :],
                    func=ACT.Copy,
                    scale=1.0 / C,
                )

            # ---------------- phase B2: conv 7x7 -> sa --------------------
            def phase_b2(b):
                mdr = dpool.tile([2, HW], F32, tag="mdr")
                for ch in range(2):
                    nc.sync.dma_start(
                        out=mdr[ch, 0:GA].rearrange("(p t) -> p t", t=NB),
                        in_=maps[0:HALF, b, ch, :],
                    )
                    nc.sync.dma_start(
                        out=mdr[ch, GA:HW].rearrange("(p t) -> p t", t=NBF),
                        in_=maps[HALF:P, b, ch, 0:NBF],
                    )
                cin = work.tile([56, 2, 56], F32, tag="cin")
                for ch in range(2):
                    nc.sync.dma_start(
                        out=cin[:, ch, :],
                        in_=mdr[ch, :].rearrange("(h w) -> h w", w=56),
                    )
                # conv: Toeplitz over h (partitions), shift over w (columns)
                pconv = psB.tile([56, 56], F32, tag="pconv")
                dwi_orders = ([3, 0, 1, 2, 4, 5, 6], [0, 1, 2, 3, 4, 5, 6])
                first = True
                for ch in range(2):
                    for dwi in dwi_orders[ch]:
                        dw = dwi - 3
                        wo0 = max(0, -dw)
                        wo1 = 56 - max(0, dw)
                        last = ch == 1 and dwi == 6
                        nc.tensor.matmul(
                            pconv[:, wo0:wo1],
                            lhsT=t_sb[:, ch * 7 + dwi, :],
                            rhs=cin[:, ch, wo0 + dw : wo1 + dw],
                            start=first,
                            stop=last,
                        )
                        first = False
                sawh = work.tile([56, 56], F32, tag="sawh")
                nc.scalar.activation(out=sawh, in_=pconv, func=ACT.Sigmoid)
                sdr = dpool.tile([HW], F32, tag="sdr")
                nc.sync.dma_start(
                    out=sdr[:].rearrange("(h w) -> h w", w=56), in_=sawh
                )
                nc.sync.dma_start(
                    out=saf[0:HALF, b, :],
                    in_=sdr[0:GA].rearrange("(p t) -> p t", t=NB),
                )
                nc.sync.dma_start(
                    out=saf[HALF:P, b, 0:NBF],
                    in_=sdr[GA:HW].rearrange("(p t) -> p t", t=NBF),
                )

            # ---------------- phase B3: apply sa + DMA out ------------------
            def phase_b3(b):
                for t in range(NB):
                    pp = _pp(t)
                    nc.gpsimd.tensor_scalar_mul(
                        out=X[0:pp, b, t, :],
                        in0=X[0:pp, b, t, :],
                        scalar1=saf[0:pp, b, t : t + 1],
                    )
                nc.sync.dma_start(
                    out=out_hwc[b, 0:GA, :].rearrange("(p t) c -> p t c", t=NB),
                    in_=X[0:HALF, b, 0:NB, :],
                )
                nc.sync.dma_start(
                    out=out_hwc[b, GA:HW, :].rearrange("(p t) c -> p t c", t=NBF),
                    in_=X[HALF:P, b, 0:NBF, :],
                )

            # ---------------- issue order ----------------------------------
            phase_a(0)
            phase_a(1)
            phase_b1(0)
            phase_a(2)
            phase_b1(1)
            phase_a(3)
            phase_b1(2)
            phase_b1(3)

            # Toeplitz bands: PE accumulates diag(cw) @ dmask per (ch, dwi)
            t_sb = cpool.tile([56, 14, 56], F32)
            for ch in range(2):
                pband = psB.tile([56, 7, 56], F32, tag="pconv")
                for dwi in range(7):
                    for dhi in range(7):
                        idx = dhi * 14 + dwi * 2 + ch
                        nc.tensor.matmul(
                            pband[:, dwi, :],
                            lhsT=diags[:, idx, :],
                            rhs=dmask_sb[:, dhi, :],
                            start=(dhi == 0),
                            stop=(dhi == 6),
                        )
                nc.scalar.copy(out=t_sb[:, ch * 7 : ch * 7 + 7, :], in_=pband)

            for b in range(NIMG):
                phase_b2(b)
            for b in range(NIMG):
                phase_b3(b)

    nc.finalize()
    return nc


LAST_RESULTS = None


def kernel(x, w1, b1, w2, b2, conv_w):
    global LAST_RESULTS
    nc = _CACHE.get("nc")
    if nc is None:
        nc = _build_nc()
        _CACHE["nc"] = nc

    x = np.ascontiguousarray(np.asarray(x, dtype=np.float32))
    shards = np.split(x, NCORES, axis=0)
    common = {
        "w1": np.ascontiguousarray(np.asarray(w1, dtype=np.float32)),
        "b1": np.ascontiguousarray(np.asarray(b1, dtype=np.float32)),
        "w2": np.ascontiguousarray(np.asarray(w2, dtype=np.float32)),
        "b2": np.ascontiguousarray(np.asarray(b2, dtype=np.float32)),
        "conv_w": np.ascontiguousarray(np.asarray(conv_w, dtype=np.float32)),
    }
    in_maps = [dict(common, x=np.ascontiguousarray(s)) for s in shards]

    res = run_bass_kernel_spmd(
        nc,
        in_maps,
        core_ids=list(range(NCORES)),
        trace=bool(int(os.environ.get("CBAM_TRACE", "0"))),
    )
    LAST_RESULTS = res
    return np.concatenate([r["out"] for r in res.results], axis=0)


# revision 16
# speedup vs baseline: 2.8040x; 2.8040x over previous
"""CBAM block (channel + spatial attention) Trainium2 Bass kernel.

Problem: x [32, 56, 56, 256] f32; data-parallel over batch across 8 NeuronCores
(4 images per core).  Everything is hardcoded for these shapes.

Per-core dataflow (B=4 images, each [3136(hw), 256(c)] f32, SBUF-resident):

  Layout: flat row n of an image maps to (partition p, block t) as
    group A: p in [0, 64),    t in [0, 25): n = 25*p + t
    group B: p in [64, 128),  t in [0, 24): n = 1600 + 24*(p-64) + t
  so each partition holds a CONTIGUOUS run of rows; every in/out DMA moves
  24-25 KB contiguous runs per partition, and the spatial maps [p, t] are
  flat-n contiguous per partition (cheap DRAM roundtrip for the conv
  rearrange).  Per-block op shapes are identical to a 128x(24)+64x1 split:
  t in [0, 24) covers all 128 partitions, t = 24 covers partitions [0, 64).

  Phase A (channel attention, per image):
    - max over hw: DVE tensor_reduce over t + half-block max -> acc [128, 256];
      2 PE transposes + DVE max over the transposed chunks -> statsT[:, :, 1]
      (channel-on-partition layout, no gpsimd all-reduce).
    - sum over hw: PE one-hot matmuls accumulated over the 25 blocks -> ACT
      scale 1/HW -> 2 PE transposes -> statsT[:, :, 0].
    - MLP W1/relu/W2, sigmoid(colA + colM + 2*b2) -> caT [128, 2] ->
      PE transpose + ones-broadcast matmul -> bca[:, b, :] [128, 256].
  Toeplitz bands (after phase A issue, consumed by the conv):
    t_sb[h_in, ch*7+dwi, h_out] = conv_w[h_in-h_out+3, dwi, ch], built on-chip:
    GPSIMD scales ident56 by each broadcast conv weight (diag matrices), PE
    accumulates diag @ dmask into PSUM per (ch, dwi) band.
  Phase B1 (per image, per block):
    - DVE tensor_tensor_reduce: xr = x * bca in place, accum max over c -> maxc.
    - sum over c -> sumc: even blocks ACT copy+accum_out, odd blocks DVE
      tensor_reduce (engine balance).
  Phase B2 (per image): maps [p, t] -> DRAM flat n -> [h, w] tiles; conv as 14
    accumulated PE matmuls (Toeplitz over h, shift over w); ACT sigmoid; back
    via DRAM flat -> saf [p, t].
  Phase B3 (per image): out = xr * sa per-partition scalar per block, split
    ACT/GPSIMD; 2 contiguous out-DMAs.
"""

import os

import numpy as np

import concourse.bass as bass
import concourse.bacc as bacc
import concourse.bass_isa as bass_isa
import concourse.tile as tile
from concourse import mybir
from concourse.bass_utils import run_bass_kernel_spmd

F32 = mybir.dt.float32
BF16 = mybir.dt.bfloat16
AX = mybir.AxisListType
OP = mybir.AluOpType
ACT = mybir.ActivationFunctionType

P = 128          # partitions
NB = 25          # blocks in group A (group B has 24)
NBF = 24         # full-width blocks
HALF = 64        # partitions in group A / valid rows in block 24
C = 256          # channels
HW = 3136        # 56*56
GA = 1600        # rows in group A (64 * 25)
NIMG = 4         # images per core
NCORES = 8

_CACHE: dict = {}


def _pp(t: int) -> int:
    return P if t < NBF else HALF


def _build_nc() -> bass.Bass:
    nc = bacc.Bacc()

    x_d = nc.dram_tensor("x", [NIMG, 56, 56, C], F32, kind="ExternalInput")
    w1_d = nc.dram_tensor("w1", [C, 16], F32, kind="ExternalInput")
    b1_d = nc.dram_tensor("b1", [16], F32, kind="ExternalInput")
    w2_d = nc.dram_tensor("w2", [16, C], F32, kind="ExternalInput")
    b2_d = nc.dram_tensor("b2", [C], F32, kind="ExternalInput")
    cw_d = nc.dram_tensor("conv_w", [7, 7, 2, 1], F32, kind="ExternalInput")
    out_d = nc.dram_tensor("out", [NIMG, 56, 56, C], BF16, kind="ExternalOutput")

    ident_d = nc.inline_tensor(np.eye(128, dtype=np.float32), name="ident128")

    # D[a, dhi, b] = 1 iff a - b == dhi - 3 (0/1 diagonal masks; the Toeplitz
    # band over h for tap dhi)
    dmask_np = np.zeros((56, 7, 56), dtype=np.float32)
    for di in range(7):
        for a in range(56):
            b = a - (di - 3)
            if 0 <= b < 56:
                dmask_np[a, di, b] = 1.0
    dmask_d = nc.inline_tensor(dmask_np, name="dmask")

    x_hwc = x_d[:].rearrange("b h w c -> b (h w) c")
    out_hwc = out_d[:].rearrange("b h w c -> b (h w) c")

    with tile.TileContext(nc) as tc:
        import contextlib

        with contextlib.ExitStack() as ctx:
            cpool = ctx.enter_context(tc.tile_pool(name="cpool", bufs=1))
            xpool = ctx.enter_context(tc.tile_pool(name="xpool", bufs=1))
            xopool = ctx.enter_context(tc.tile_pool(name="xopool", bufs=2))
            work = ctx.enter_context(tc.tile_pool(name="work", bufs=3))
            small = ctx.enter_context(tc.tile_pool(name="small", bufs=3))
            psA = ctx.enter_context(tc.tile_pool(name="psA", bufs=2, space="PSUM"))
            psB = ctx.enter_context(tc.tile_pool(name="psB", bufs=2, space="PSUM"))
            psM = ctx.enter_context(tc.tile_pool(name="psM", bufs=2, space="PSUM"))
            dpool = ctx.enter_context(tc.tile_pool(name="dpool", bufs=2, space="DRAM"))

            # ---------------- constants & weights ----------------
            ident = cpool.tile([128, 128], F32)
            nc.sync.dma_start(out=ident, in_=ident_d[:])

            w1_sb = cpool.tile([128, 2, 16], F32)
            nc.sync.dma_start(out=w1_sb, in_=w1_d[:].rearrange("(j p) m -> p j m", p=128))
            w2_sb = cpool.tile([16, 2, 128], F32)
            nc.sync.dma_start(out=w2_sb, in_=w2_d[:].rearrange("k (j m) -> k j m", j=2))
            b1_sb = cpool.tile([16, 1], F32)
            nc.sync.dma_start(out=b1_sb, in_=b1_d[:].rearrange("(p o) -> p o", o=1))
            b2_sb = cpool.tile([128, 2], F32)
            nc.sync.dma_start(out=b2_sb, in_=b2_d[:].rearrange("(j p) -> p j", p=128))
            b2x2 = cpool.tile([128, 2], F32)
            nc.scalar.activation(out=b2x2, in_=b2_sb, func=ACT.Copy, scale=2.0)

            oh2 = cpool.tile([128, 2], F32)
            nc.vector.memset(oh2[:, 0:1], 1.0)
            nc.vector.memset(oh2[:, 1:2], 0.0)
            ones_r = cpool.tile([1, 128], F32)
            nc.vector.memset(ones_r, 1.0)

            dmask_sb = cpool.tile([56, 7, 56], F32)
            nc.sync.dma_start(out=dmask_sb, in_=dmask_d[:])
            cw_row = cpool.tile([1, 98], F32)
            nc.sync.dma_start(
                out=cw_row, in_=cw_d[:].rearrange("a b c o -> o (a b c)")
            )
            # broadcast the 98 conv weights to 56 partitions
            pcw = psM.tile([56, 98], F32, tag="mlp")
            nc.tensor.matmul(
                pcw, lhsT=ones_r[:, 0:56], rhs=cw_row, start=True, stop=True
            )
            cwb = cpool.tile([56, 98], F32)
            nc.scalar.copy(out=cwb, in_=pcw)

            # ---------------- big SBUF state ----------------
            X = xpool.tile([P, NIMG, NB, C], F32)
            bca = cpool.tile([P, NIMG, C], F32)
            maps = cpool.tile([P, NIMG, 2, NB], F32)  # [., ., 0=sumc 1=maxc, .]
            saf = cpool.tile([P, NIMG, NB], F32)
            scr = cpool.tile([P, C], F32)  # ACT dummy-copy target for accum

            # ACT: build the 98 diag(conv_w) matrices early — fills ScalarE's
            # otherwise-idle window during the input DMA (consumed by the
            # Toeplitz PE matmuls below)
            diags = cpool.tile([56, 98, 56], F32)
            for i in range(98):
                nc.scalar.activation(
                    out=diags[:, i, :],
                    in_=ident[0:56, 0:56],
                    func=ACT.Copy,
                    scale=cwb[:, i : i + 1],
                )

            # PE warm-up matmuls touching constant lhsT sources so that later
            # matmuls don't accumulate one sync-wait per constant tensor.
            pwu = psM.tile([128, 4], F32, tag="mlp")
            nc.tensor.matmul(pwu[0:2, 0:2], lhsT=oh2, rhs=oh2, start=True, stop=True)
            nc.tensor.matmul(
                pwu[0:4, 0:4],
                lhsT=ident[:, 0:4],
                rhs=ident[:, 0:4],
                start=True,
                stop=True,
            )
            nc.tensor.matmul(
                pwu[0:128, 0:1],
                lhsT=ones_r.rearrange("p m -> p m"),
                rhs=ones_r[:, 0:1],
                start=True,
                stop=True,
            )
            nc.tensor.matmul(
                pwu[0:4, 0:4],
                lhsT=dmask_sb[:, 0, 0:4],
                rhs=dmask_sb[:, 0, 0:4],
                start=True,
                stop=True,
            )
            nc.tensor.matmul(
                pwu[0:4, 0:4],
                lhsT=w1_sb[:, 0, 0:4],
                rhs=w1_sb[:, 0, 0:4],
                start=True,
                stop=True,
            )
            nc.tensor.matmul(
                pwu[0:4, 0:4],
                lhsT=w2_sb[:, 0, 0:4],
                rhs=w2_sb[:, 0, 0:4],
                start=True,
                stop=True,
            )

            # ---------------- DMA in (contiguous runs per partition) --------
            for b in range(NIMG):
                nc.sync.dma_start(
                    out=X[0:HALF, b, 0:NB, :],
                    in_=x_hwc[b, 0:GA, :].rearrange("(p t) c -> p t c", t=NB),
                )
                nc.sync.dma_start(
                    out=X[HALF:P, b, 0:NBF, :],
                    in_=x_hwc[b, GA:HW, :].rearrange("(p t) c -> p t c", t=NBF),
                )

            # Toeplitz bands: PE accumulates diag(cw) @ dmask per (ch, dwi);
            # runs during the input-DMA window (PE idle, diags stream in)
            t_sb = cpool.tile([56, 14, 56], F32)
            for ch in range(2):
                pband = psB.tile([56, 7, 56], F32, tag="pconv")
                for dwi in range(7):
                    for dhi in range(7):
                        idx = dhi * 14 + dwi * 2 + ch
                        nc.tensor.matmul(
                            pband[:, dwi, :],
                            lhsT=diags[:, idx, :],
                            rhs=dmask_sb[:, dhi, :],
                            start=(dhi == 0),
                            stop=(dhi == 6),
                        )
                nc.scalar.copy(out=t_sb[:, ch * 7 : ch * 7 + 7, :], in_=pband)

            # ---------------- phase A (channel attention) per image ---------
            def phase_a(b):
                # max over hw: contiguous log-tree of pairwise maxes over the
                # 24 full blocks (3D APs, c innermost), then the half block;
                # cross-partition via PE transposes + DVE max afterwards
                m12 = work.tile([P, 12, C], F32, tag="m12", bufs=1)
                m6 = work.tile([P, 6, C], F32, tag="m6", bufs=1)
                m3 = work.tile([P, 3, C], F32, tag="m3", bufs=1)
                acc = work.tile([P, C], F32, tag="acc")
                nc.vector.tensor_max(
                    out=m12, in0=X[:, b, 0:12, :], in1=X[:, b, 12:NBF, :]
                )
                nc.vector.tensor_max(
                    out=m6, in0=m12[:, 0:6, :], in1=m12[:, 6:12, :]
                )
                nc.vector.tensor_max(
                    out=m3, in0=m6[:, 0:3, :], in1=m6[:, 3:6, :]
                )
                nc.vector.tensor_max(
                    out=acc, in0=m3[:, 0, :], in1=m3[:, 1, :]
                )
                nc.vector.tensor_max(out=acc, in0=acc, in1=m3[:, 2, :])
                nc.vector.tensor_max(
                    out=acc[0:HALF], in0=acc[0:HALF], in1=X[0:HALF, b, NBF, :]
                )
                pmaxT = psM.tile([128, 2, 128], F32, tag="mlp")
                for j in range(2):
                    nc.tensor.transpose(
                        pmaxT[:, j, :], acc[:, j * 128 : (j + 1) * 128], ident
                    )
                statsT = small.tile([128, 2, 2], F32, tag="statsT")
                nc.vector.tensor_reduce(
                    out=statsT[:, :, 1:2], in_=pmaxT, axis=AX.X, op=OP.max
                )

                # sum over hw on PE (lands on psum partition row 0)
                ps = psA.tile([2, C], F32, tag="ps_sum")
                for t in range(NBF):
                    nc.tensor.matmul(
                        ps, lhsT=oh2, rhs=X[:, b, t, :], start=(t == 0), stop=False
                    )
                nc.tensor.matmul(
                    ps,
                    lhsT=oh2[0:HALF],
                    rhs=X[0:HALF, b, NBF, :],
                    start=False,
                    stop=True,
                )
                savg = small.tile([1, C], F32, tag="savg")
                nc.scalar.activation(
                    out=savg, in_=ps[0:1, :], func=ACT.Copy, scale=1.0 / HW
                )
                pavgT = psM.tile([128, 2, 1], F32, tag="mlp")
                for j in range(2):
                    nc.tensor.transpose(
                        pavgT[:, j, :],
                        savg[:, j * 128 : (j + 1) * 128],
                        ident[0:1, 0:1],
                    )
                nc.scalar.copy(out=statsT[:, :, 0:1], in_=pavgT)

                # MLP layer 1: h = relu(W1^T statsT + b1)
                ph = psM.tile([16, 2], F32, tag="mlp")
                for j in range(2):
                    nc.tensor.matmul(
                        ph,
                        lhsT=w1_sb[:, j, :],
                        rhs=statsT[:, j, :],
                        start=(j == 0),
                        stop=(j == 1),
                    )
                h_sb = small.tile([16, 2], F32, tag="h_sb")
                nc.scalar.activation(
                    out=h_sb, in_=ph, func=ACT.Relu, bias=b1_sb, scale=1.0
                )

                # layer 2 + combine + sigmoid -> caT [256] in 2 chunks
                caT = small.tile([128, 2], F32, tag="caT")
                for j in range(2):
                    pc = psM.tile([128, 2], F32, tag="mlp")
                    nc.tensor.matmul(
                        pc, lhsT=w2_sb[:, j, :], rhs=h_sb, start=True, stop=True
                    )
                    pc_sb = small.tile([128, 2], F32, tag="pc_sb")
                    nc.scalar.copy(out=pc_sb, in_=pc)
                    catmp = small.tile([128, 1], F32, tag="catmp")
                    nc.vector.tensor_add(
                        out=catmp, in0=pc_sb[:, 0:1], in1=pc_sb[:, 1:2]
                    )
                    nc.scalar.activation(
                        out=caT[:, j : j + 1],
                        in_=catmp,
                        func=ACT.Sigmoid,
                        bias=b2x2[:, j : j + 1],
                        scale=1.0,
                    )

                # broadcast ca over partitions: bca[:, b, :]
                pcr = psM.tile([1, 2, 128], F32, tag="mlp")
                for j in range(2):
                    nc.tensor.transpose(pcr[:, j, :], caT[:, j : j + 1], ident)
                ca_row = small.tile([1, 256], F32, tag="ca_row")
                nc.scalar.copy(out=ca_row, in_=pcr.rearrange("p j m -> p (j m)"))
                pbca = psB.tile([P, C], F32, tag="pbca")
                nc.tensor.matmul(pbca, lhsT=ones_r, rhs=ca_row, start=True, stop=True)
                nc.scalar.copy(out=bca[:, b, :], in_=pbca)

            # ---------------- phase B1: xr = x * ca, spatial stats ----------
            def phase_b1(b):
                for t in range(NB):
                    pp = _pp(t)
                    nc.vector.tensor_mul(
                        out=X[0:pp, b, t, :],
                        in0=X[0:pp, b, t, :],
                        in1=bca[0:pp, b, :],
                    )
                    nc.scalar.activation(
                        out=scr[0:pp, :],
                        in_=X[0:pp, b, t, :],
                        func=ACT.Copy,
                        accum_out=maps[0:pp, b, 0, t : t + 1],
                    )
                # max over c: one contiguous 3D reduce for the 24 full
                # blocks + one for the group-A extra block
                nc.vector.tensor_reduce(
                    out=maps[:, b, 1, 0:NBF],
                    in_=X[:, b, 0:NBF, :],
                    axis=AX.X,
                    op=OP.max,
                )
                nc.vector.tensor_reduce(
                    out=maps[0:HALF, b, 1, NBF : NBF + 1],
                    in_=X[0:HALF, b, NBF : NBF + 1, :],
                    axis=AX.X,
                    op=OP.max,
                )
                # mean = sum / C
                nc.scalar.activation(
                    out=maps[:, b, 0, :],
                    in_=maps[:, b, 0, :],
                    func=ACT.Copy,
                    scale=1.0 / C,
                )

            # ---------------- phase B2: conv 7x7 -> sa --------------------
            def phase_b2(b):
                mdr = dpool.tile([2, HW], F32, tag="mdr")
                for ch in range(2):
                    nc.scalar.dma_start(
                        out=mdr[ch, 0:GA].rearrange("(p t) -> p t", t=NB),
                        in_=maps[0:HALF, b, ch, :],
                    )
                    nc.scalar.dma_start(
                        out=mdr[ch, GA:HW].rearrange("(p t) -> p t", t=NBF),
                        in_=maps[HALF:P, b, ch, 0:NBF],
                    )
                cin = work.tile([56, 2, 56], F32, tag="cin")
                for ch in range(2):
                    nc.scalar.dma_start(
                        out=cin[:, ch, :],
                        in_=mdr[ch, :].rearrange("(h w) -> h w", w=56),
                    )
                # conv: Toeplitz over h (partitions), shift over w (columns)
                pconv = psB.tile([56, 56], F32, tag="pconv")
                dwi_orders = ([3, 0, 1, 2, 4, 5, 6], [0, 1, 2, 3, 4, 5, 6])
                first = True
                for ch in range(2):
                    for dwi in dwi_orders[ch]:
                        dw = dwi - 3
                        wo0 = max(0, -dw)
                        wo1 = 56 - max(0, dw)
                        last = ch == 1 and dwi == 6
                        nc.tensor.matmul(
                            pconv[:, wo0:wo1],
                            lhsT=t_sb[:, ch * 7 + dwi, :],
                            rhs=cin[:, ch, wo0 + dw : wo1 + dw],
                            start=first,
                            stop=last,
                        )
                        first = False
                sawh = work.tile([56, 56], F32, tag="sawh")
                nc.scalar.activation(out=sawh, in_=pconv, func=ACT.Sigmoid)
                sdr = dpool.tile([HW], F32, tag="sdr")
                nc.scalar.dma_start(
                    out=sdr[:].rearrange("(h w) -> h w", w=56), in_=sawh
                )
                nc.scalar.dma_start(
                    out=saf[0:HALF, b, :],
                    in_=sdr[0:GA].rearrange("(p t) -> p t", t=NB),
                )
                nc.scalar.dma_start(
                    out=saf[HALF:P, b, 0:NBF],
                    in_=sdr[GA:HW].rearrange("(p t) -> p t", t=NBF),
                )

            # ---------------- phase B3: apply sa + DMA out ------------------
            def phase_b3(b):
                xo = xopool.tile([P, NB, C], BF16, tag="xo")
                for t in range(NB):
                    pp = _pp(t)
                    if t % 2 == 0:
                        nc.vector.tensor_scalar_mul(
                            out=xo[0:pp, t, :],
                            in0=X[0:pp, b, t, :],
                            scalar1=saf[0:pp, b, t : t + 1],
                        )
                    else:
                        nc.scalar.activation(
                            out=xo[0:pp, t, :],
                            in_=X[0:pp, b, t, :],
                            func=ACT.Copy,
                            scale=saf[0:pp, b, t : t + 1],
                        )
                nc.sync.dma_start(
                    out=out_hwc[b, 0:GA, :].rearrange("(p t) c -> p t c", t=NB),
                    in_=xo[0:HALF, 0:NB, :],
                )
                nc.sync.dma_start(
                    out=out_hwc[b, GA:HW, :].rearrange("(p t) c -> p t c", t=NBF),
                    in_=xo[HALF:P, 0:NBF, :],
                )

            # ---------------- issue order ----------------------------------
            phase_a(0)
            phase_a(1)
            phase_b1(0)
            phase_a(2)
            phase_b2(0)
            phase_b1(1)
            phase_a(3)
            phase_b3(0)
            phase_b2(1)
            phase_b1(2)
            phase_b3(1)
            phase_b2(2)
            phase_b1(3)
            phase_b3(2)
            phase_b2(3)
            phase_b3(3)

    nc.finalize()
    return nc


LAST_RESULTS = None


def kernel(x, w1, b1, w2, b2, conv_w):
    global LAST_RESULTS
    nc = _CACHE.get("nc")
    if nc is None:
        nc = _build_nc()
        _CACHE["nc"] = nc

    x = np.ascontiguousarray(np.asarray(x, dtype=np.float32))
    shards = np.split(x, NCORES, axis=0)
    common = {
        "w1": np.ascontiguousarray(np.asarray(w1, dtype=np.float32)),
        "b1": np.ascontiguousarray(np.asarray(b1, dtype=np.float32)),
        "w2": np.ascontiguousarray(np.asarray(w2, dtype=np.float32)),
        "b2": np.ascontiguousarray(np.asarray(b2, dtype=np.float32)),
        "conv_w": np.ascontiguousarray(np.asarray(conv_w, dtype=np.float32)),
    }
    in_maps = [dict(common, x=np.ascontiguousarray(s)) for s in shards]

    res = run_bass_kernel_spmd(
        nc,
        in_maps,
        core_ids=list(range(NCORES)),
        trace=bool(int(os.environ.get("CBAM_TRACE", "0"))),
    )
    LAST_RESULTS = res
    return np.concatenate(
        [np.asarray(r["out"]).astype(np.float32) for r in res.results], axis=0
    )
